# revision 1
# baseline (speedup 1.0000x reference)
"""GAT forward kernel for Trainium2 (8 NeuronCores, Bass/Tile).

Reference computation (dense form):
    adj = densify(A); Wh = X @ Ws; e = leaky_relu(Wh@a1 + (Wh@a2).T, 0.15)
    att = softmax(where(adj>0, e, -9e15), axis=1); out = elu(att @ Wh)

This kernel exploits sparsity: only ~524K edges out of 16384^2 matter.
Because |e| <= ~16 (bounded inputs), softmax needs no max-subtraction:
    w_e = exp(leaky(s_src + t_dst));  out_i = elu(sum_e w_e Wh_dst / sum_e w_e)
with exp(-9e15) == 0 handled by simply not summing non-edges, and duplicate
edges deduplicated on the host (reference only uses adj > 0).

Sharding: rows (softmax queries) split 2048/core across 8 cores. Each core:
  P1: computes Wh = X @ Ws (PE transpose + matmul), s = Wh@a1, t = Wh@a2
      for ALL nodes, writes a DRAM table row j = [t_j f32, s_j f32, Wh_j bf16] (256B).
  P3: dma_gathers table rows by edge dst (512B) and the 256B head window by
      edge src (elem_step trick) for its own edges; computes w on DVE/ACT;
      segment-aggregates per 128-row block via one-hot PE matmuls:
          acc[128,65] += onehot(srcrel)[128e,128r].T @ (w * [Wh_dst, 1])
      then out = elu(U / Z) and writes its 2048 output rows.

Host prep packs edges into per-(core,block) buckets padded to a cross-core
uniform chunk count so all 8 cores run the same program (SPMD).
"""
import os
import sys

if "/opt/trn_rl_repo" not in sys.path:
    sys.path.insert(0, "/opt/trn_rl_repo")

_ABL = set(os.environ.get("GAT_ABLATE", "").split(","))

from contextlib import ExitStack

import numpy as np

import concourse.bass as bass
import concourse.tile as tile
from concourse import bacc, mybir
from concourse.bass_utils import run_bass_kernel_spmd
from concourse.masks import make_identity

N = 16384          # nodes
F = 128            # input features
D = 64             # embedding dim
NCORES = 8
R = N // NCORES    # rows per core (2048)
NB = R // 128      # row blocks per core (16)
NBG = N // 128     # global node blocks (128)
TW = 64            # table row width in f32 slots (256 bytes)
dt = mybir.dt


# ---------------------------------------------------------------- host prep
def _prep_edges(A):
    """Dedup edges, bucket by (core, block) with each row's edges padded to a
    multiple of 16 (so every 16-slot "cell" belongs to one src row), then pad
    blocks to cross-core uniform chunk counts Kb. Returns per-core index /
    srcrel / cell arrays and the shared Kb."""
    src_all = np.asarray(A[0], dtype=np.int64)
    dst_all = np.asarray(A[1], dtype=np.int64)
    keys = np.unique(src_all * N + dst_all)     # dedup + sort by (src, dst)
    src = (keys // N).astype(np.int32)
    dst = (keys % N).astype(np.int32)

    deg = np.bincount(src, minlength=N)
    assert deg.min() > 0, (
        "empty rows present; this kernel assumes every row has >=1 edge"
    )
    deg16 = ((deg + 15) // 16) * 16              # 16-aligned row sizes
    gb = np.arange(N) >> 7
    cnt16 = np.bincount(gb, weights=deg16, minlength=NBG).astype(np.int64)
    cnt16 = cnt16.reshape(NCORES, NB)
    Kb = np.maximum((cnt16.max(axis=0) + 127) // 128, 1)          # [NB]
    S = int(Kb.sum()) * 128                      # slots per core
    offs = np.concatenate([[0], np.cumsum(Kb)]) * 128  # slot offset per block
    # cells per block, padded to 128-cell granularity for the gather
    ncell = [int(k) * 8 for k in Kb]
    ncellp = [((n + 127) // 128) * 128 for n in ncell]
    cell_offs = np.concatenate([[0], np.cumsum(ncellp)])
    SC = int(cell_offs[-1])                      # padded cells per core

    row_start = np.concatenate([[0], np.cumsum(deg)])

    dsti = np.zeros((NCORES, S), np.int16)       # table idx for dst gather
    srcrel = np.full((NCORES, S), -1.0, np.float32)  # row-in-block, -1 = pad
    cellsrc = np.zeros((NCORES, SC), np.int16)   # global src row per cell
    for c in range(NCORES):
        for b in range(NB):
            rows = np.arange((c * NB + b) * 128, (c * NB + b) * 128 + 128)
            pos = offs[b]
            for r in rows:
                d = int(deg[r])
                lo = row_start[r]
                dsti[c, pos:pos + d] = dst[lo:lo + d]
                srcrel[c, pos:pos + d] = float(r & 127)
                nc16 = int(deg16[r])
                cbase = cell_offs[b] + (pos - offs[b]) // 16
                cellsrc[c, cbase:cbase + nc16 // 16] = r
                pos += nc16
            assert pos <= offs[b + 1]

    def wrap(x):
        n = x.shape[0]
        w = x.reshape(n // 16, 16).T             # [16, n/16]
        return np.tile(w, (8, 1)).copy()         # [128, n/16]

    cores = []
    for c in range(NCORES):
        cores.append({
            "dsti": wrap(dsti[c]),
            "celli": wrap(cellsrc[c]),
            "srcrel": srcrel[c].reshape(S // 128, 128).T.copy(),  # [128, S/128]
        })
    return cores, [int(k) for k in Kb], S, [int(x) for x in ncellp]


_qctr = [0]


def _q():
    # Strict issue-order rotation over the 4 SWDGE queues: Tile assigns DMA-SW
    # sem lanes round-robin (k % 8), so queue = k % 4 keeps every lane pinned
    # to one queue (8 % 4 == 0).
    return 0


# ---------------------------------------------------------------- device IR
def _build(Kb, S, ncellp):
    _qctr[0] = 0
    SC = sum(ncellp)
    nc = bacc.Bacc("TRN2", target_bir_lowering=False, debug=False,
                   enable_asserts=False, num_devices=NCORES,
                   num_swdge_queues=4)
    X_d = nc.dram_tensor("X", [N, F], dt.float32, kind="ExternalInput").ap()
    Ws_d = nc.dram_tensor("Ws", [F, D], dt.float32, kind="ExternalInput").ap()
    a1_d = nc.dram_tensor("a1b", [128, D], dt.float32, kind="ExternalInput").ap()
    a2_d = nc.dram_tensor("a2b", [128, D], dt.float32, kind="ExternalInput").ap()
    dsti_d = nc.dram_tensor("dsti", [128, S // 16], dt.int16, kind="ExternalInput").ap()
    celli_d = nc.dram_tensor("celli", [128, SC // 16], dt.int16, kind="ExternalInput").ap()
    srel_d = nc.dram_tensor("srcrel", [128, S // 128], dt.float32, kind="ExternalInput").ap()
    sel8_d = nc.dram_tensor("sel8", [128, 16], dt.float32, kind="ExternalInput").ap()
    E8_d = nc.dram_tensor("E8", [128, 128], dt.float32, kind="ExternalInput").ap()
    out_d = nc.dram_tensor("out", [R, D], dt.float32, kind="ExternalOutput").ap()

    with tile.TileContext(nc) as tc, ExitStack() as ctx:
        cpool = ctx.enter_context(tc.tile_pool(name="const", bufs=1))
        dram = ctx.enter_context(tc.tile_pool(name="dram", bufs=1, space="DRAM"))
        xpool = ctx.enter_context(tc.tile_pool(name="x", bufs=2))
        xtpool = ctx.enter_context(tc.tile_pool(name="xt", bufs=2))
        twpool = ctx.enter_context(tc.tile_pool(name="tw", bufs=2))
        tmppool = ctx.enter_context(tc.tile_pool(name="tmp", bufs=2))
        # PSUM budget (8 banks): big pool 2x2 banks shared by P1 transposes
        # and nothing else; small pool 2x1 shared by P1 Wh and P3 acc;
        # s-expand pool 2x1.
        ps_big = ctx.enter_context(tc.tile_pool(name="ps_big", bufs=2, space="PSUM"))
        ps_sm = ctx.enter_context(tc.tile_pool(name="ps_sm", bufs=2, space="PSUM"))
        ps_se = ctx.enter_context(tc.tile_pool(name="ps_se", bufs=2, space="PSUM"))
        gpool = ctx.enter_context(tc.tile_pool(name="gat", bufs=4))
        spool = ctx.enter_context(tc.tile_pool(name="sg", bufs=4))
        wpool = ctx.enter_context(tc.tile_pool(name="w", bufs=2))
        Gpool = ctx.enter_context(tc.tile_pool(name="G", bufs=3))
        ohpool = ctx.enter_context(tc.tile_pool(name="oh", bufs=2))
        epool = ctx.enter_context(tc.tile_pool(name="ep", bufs=2))

        tabTW = dram.tile([N, TW], dt.float32)   # [t, s, Wh(64), garbage pad]

        # ---- constants
        ident = cpool.tile([128, 128], dt.float32)
        make_identity(nc, ident[:])
        iota_i = cpool.tile([128, 128], dt.int32)
        nc.gpsimd.iota(iota_i[:], pattern=[[1, 128]], base=0, channel_multiplier=0)
        iota_f = cpool.tile([128, 128], dt.float32)
        nc.vector.tensor_copy(iota_f[:], iota_i[:])
        ws_t = cpool.tile([F, D], dt.float32)
        nc.sync.dma_start(ws_t[:], Ws_d)
        a1_t = cpool.tile([128, D], dt.float32)
        nc.sync.dma_start(a1_t[:], a1_d)
        a2_t = cpool.tile([128, D], dt.float32)
        nc.sync.dma_start(a2_t[:], a2_d)
        dsti_t = cpool.tile([128, S // 16], dt.int16)
        nc.sync.dma_start(dsti_t[:], dsti_d)
        celli_t = cpool.tile([128, SC // 16], dt.int16)
        nc.sync.dma_start(celli_t[:], celli_d)
        srel_t = cpool.tile([128, S // 128], dt.float32)
        nc.sync.dma_start(srel_t[:], srel_d)
        sel8_t = cpool.tile([128, 16], dt.float32)
        nc.sync.dma_start(sel8_t[:], sel8_d)
        E8_t = cpool.tile([128, 128], dt.float32)
        nc.sync.dma_start(E8_t[:], E8_d)

        # ---- P1: build table row j = [t_j, s_j, Wh_j(64), pad] for all nodes
        X_v = X_d.rearrange("(q k p) f -> q p k f", p=128, k=8)   # [16, 128, 8, F]
        tab_v = tabTW[:].rearrange("(q k p) w -> p q k w", p=128, k=8)
        for q in range(16):                       # groups of 8 node blocks
            xb8 = xpool.tile([128, 8, F], dt.float32)
            nc.scalar.dma_start(xb8[:], X_v[q])
            tw = twpool.tile([128, 8, 34], dt.float32)
            xt8_ps = ps_big.tile([128, 8, 128], dt.float32, space="PSUM", tag="big")
            if "nop1" not in _ABL:
                for k in range(8):
                    nc.tensor.transpose(xt8_ps[:, k, :], xb8[:, k, :], ident[:])
                xt8 = xtpool.tile([128, 8, 128], dt.float32)
                nc.vector.tensor_copy(xt8[:], xt8_ps[:])
                wh_ps8 = ps_sm.tile([128, 8, D], dt.float32, space="PSUM", tag="sm")
                for k in range(8):
                    nc.tensor.matmul(wh_ps8[:, k, :], lhsT=xt8[:, k, :],
                                     rhs=ws_t[:], start=True, stop=True)
                nc.vector.tensor_copy(tw[:, :, 2:34].bitcast(dt.bfloat16), wh_ps8[:])
                tmp = tmppool.tile([128, 8, D], dt.float32)
                nc.vector.tensor_mul(
                    tmp[:], wh_ps8[:],
                    a2_t[:, None, :].to_broadcast([128, 8, D]))
                nc.vector.reduce_sum(tw[:, :, 0:1], tmp[:],
                                     axis=mybir.AxisListType.X)
                tmp2 = tmppool.tile([128, 8, D], dt.float32)
                nc.vector.tensor_mul(
                    tmp2[:], wh_ps8[:],
                    a1_t[:, None, :].to_broadcast([128, 8, D]))
                nc.vector.reduce_sum(tw[:, :, 1:2], tmp2[:],
                                     axis=mybir.AxisListType.X)
            # write rows (q*8+k)*128+p, cols 0:34 (pad cols stay garbage --
            # they are gathered but never read by any compute)
            nc.scalar.dma_start(tab_v[:, q, :, 0:34], tw[:])

        # ---- P3: per-block gather + weight + one-hot aggregate + epilogue
        tab_ap = tabTW[:]                                    # [N, 128] rows
        outstage = cpool.tile([128, NB, D], dt.float32)
        off = 0
        cell_off = 0
        for b in range(NB):
            K = Kb[b]
            n_idx = K * 128
            ncp = ncellp[b]                      # padded cell count (x128)
            nm = ncp // 128                      # 16-chunk spans
            # dma_gather is limited to 1024 indices per call (64 descriptors
            # per SDMA engine, single packet) -- split into 8-chunk sub-calls.
            gat = gpool.tile([128, K, TW], dt.float32)
            if "init" in _ABL:
                nc.vector.memzero(gat[:])
            for c0 in range(0, K, 8):
                nch = min(8, K - c0)
                ni = nch * 128
                o = off + c0 * 128
                if "nogat" not in _ABL:
                    nc.gpsimd.dma_gather(
                        out_ap=gat[:, c0:c0 + nch, :], in_ap=tab_ap,
                        idxs_ap=dsti_t[:, o // 16:(o + ni) // 16],
                        num_idxs=ni, num_idxs_reg=ni, elem_size=TW,
                        queue_num=_q(),
                    )
            # s per cell (one row per 16 aligned slots), then expand to the
            # edge layout via a constant matmul:
            #   s_edge[p, 16m+cl] = cellval[8*cl + p//16, m]
            #                     = sum_q E8[q, p] * (cellval[q, m] * sel8[q, cl])
            cellv = spool.tile([128, nm, 64], dt.float32)
            if "nosg" not in _ABL:
                nc.gpsimd.dma_gather(
                    out_ap=cellv[:], in_ap=tab_ap,
                    idxs_ap=celli_t[:, cell_off // 16:(cell_off + ncp) // 16],
                    num_idxs=ncp, num_idxs_reg=ncp, elem_size=TW,
                    queue_num=_q(),
                )
            else:
                nc.vector.memzero(cellv[:])
            s_ps = ps_se.tile([128, nm * 16], dt.float32, space="PSUM", tag="se")
            for m in range(nm):
                rhsm = wpool.tile([128, 16], dt.float32, tag="rhsm")
                nc.vector.tensor_mul(
                    rhsm[:], sel8_t[:],
                    cellv[:, m, 1:2].to_broadcast([128, 16]))
                nc.tensor.matmul(s_ps[:, m * 16:(m + 1) * 16], lhsT=E8_t[:],
                                 rhs=rhsm[:], start=True, stop=True)
            # w = exp(leaky(s + t))
            e_t = wpool.tile([128, K], dt.float32, tag="e")
            nc.vector.tensor_add(e_t[:], s_ps[:, 0:K], gat[:, :, 0])
            lk = wpool.tile([128, K], dt.float32, tag="lk")
            nc.vector.scalar_tensor_tensor(
                out=lk[:], in0=e_t[:], scalar=0.15, op0=mybir.AluOpType.mult,
                in1=e_t[:], op1=mybir.AluOpType.max)
            w_t = wpool.tile([128, K], dt.float32, tag="wt")
            nc.scalar.activation(w_t[:], lk[:], mybir.ActivationFunctionType.Exp)
            # G = [w * Wh_dst, w]
            G = Gpool.tile([128, K, D + 1], dt.float32)
            nc.vector.tensor_mul(G[:, :, 0:D], gat[:, :, 2:34].bitcast(dt.bfloat16),
                                 w_t[:, :, None].to_broadcast([128, K, D]))
            nc.vector.tensor_copy(G[:, :, D], w_t[:])
            # one-hot of srcrel vs row-in-block
            oh = ohpool.tile([128, K, 128], dt.float32)
            if "nooh" in _ABL:
                nc.vector.memzero(oh[:, 0, :])
            else:
                nc.vector.tensor_tensor(
                    out=oh[:],
                    in0=iota_f[:, None, :].to_broadcast([128, K, 128]),
                    in1=srel_t[:, off // 128:off // 128 + K, None]
                        .to_broadcast([128, K, 128]),
                    op=mybir.AluOpType.is_equal)
            # aggregate
            acc = ps_sm.tile([128, D + 1], dt.float32, space="PSUM", tag="sm")
            nmm = 1 if "nomm" in _ABL else K
            for c in range(nmm):
                nc.tensor.matmul(acc[:], lhsT=oh[:, c, :], rhs=G[:, c, :],
                                 start=(c == 0), stop=(c == nmm - 1))
            # epilogue: out = elu(U / Z)
            zg = epool.tile([128, 1], dt.float32, tag="zg")
            nc.vector.tensor_scalar_max(zg[:], acc[:, D:D + 1], 1e-30)
            zr = epool.tile([128, 1], dt.float32, tag="zr")
            nc.vector.reciprocal(zr[:], zg[:])
            x = epool.tile([128, D], dt.float32, tag="x")
            nc.vector.tensor_scalar_mul(x[:], acc[:, 0:D], zr[:])
            mn = epool.tile([128, D], dt.float32, tag="mn")
            nc.vector.tensor_scalar_min(mn[:], x[:], 0.0)
            em = epool.tile([128, D], dt.float32, tag="em")
            nc.scalar.activation(em[:], mn[:], mybir.ActivationFunctionType.Exp)
            rl = epool.tile([128, D], dt.float32, tag="rl")
            nc.vector.tensor_scalar_max(rl[:], x[:], 0.0)
            nc.vector.scalar_tensor_tensor(
                out=outstage[:, b, :], in0=em[:], scalar=-1.0,
                op0=mybir.AluOpType.add, in1=rl[:], op1=mybir.AluOpType.add)
            off += n_idx
            cell_off += ncp

        out_v = out_d.rearrange("(b p) d -> p b d", p=128)   # [128, NB, D]
        nc.sync.dma_start(out_v, outstage[:])
    nc.compile()
    return nc


_cache = {}


def _get_program(Kb, S, ncellp):
    key = (tuple(Kb), S, tuple(ncellp), tuple(sorted(_ABL)))
    if key not in _cache:
        _cache[key] = _build(Kb, S, ncellp)
    return _cache[key]


def make_in_maps(A, X, Ws, a):
    """Host-side sharding: returns (nc, in_maps)."""
    X = np.ascontiguousarray(np.asarray(X, dtype=np.float32))
    Ws = np.ascontiguousarray(np.asarray(Ws, dtype=np.float32))
    a = np.asarray(a, dtype=np.float32).reshape(2 * D)
    a1b = np.tile(a[:D][None, :], (128, 1)).astype(np.float32)
    a2b = np.tile(a[D:][None, :], (128, 1)).astype(np.float32)
    q = np.arange(128)
    sel8 = (q[:, None] // 8 == np.arange(16)[None, :]).astype(np.float32)
    E8 = (q[:, None] % 8 == q[None, :] // 16).astype(np.float32)
    cores, Kb, S, ncellp = _prep_edges(A)
    nc = _get_program(Kb, S, ncellp)
    in_maps = [
        {"X": X, "Ws": Ws, "a1b": a1b, "a2b": a2b, "sel8": sel8, "E8": E8,
         "dsti": c["dsti"], "celli": c["celli"], "srcrel": c["srcrel"]}
        for c in cores
    ]
    return nc, in_maps


def kernel(A, X, Ws, a):
    nc, in_maps = make_in_maps(A, X, Ws, a)
    res = run_bass_kernel_spmd(nc, in_maps, core_ids=list(range(NCORES)),
                               trace=False)
    return np.concatenate([r["out"] for r in res.results], axis=0)



# revision 54
# speedup vs baseline: 11737.9070x; 11737.9070x over previous
"""GAT forward kernel for Trainium2 (8 NeuronCores, Bass/Tile).

Reference computation (dense form):
    adj = densify(A); Wh = X @ Ws; e = leaky_relu(Wh@a1 + (Wh@a2).T, 0.15)
    att = softmax(where(adj>0, e, -9e15), axis=1); out = elu(att @ Wh)

Sparse formulation: only ~524K edges matter; |e| <= ~20 so softmax needs no
max-subtraction:  w_e = exp(leaky(s_src + t_dst));  out_i = elu(U_i / Z_i),
U_i = sum_e w_e Wh_dst, Z_i = sum_e w_e.  s = X@(Ws@a1), t = X@(Ws@a2).

Sharding: softmax rows split 2048/core across 8 cores (SPMD, no
collectives; the host pads edge slots to cross-core uniform chunk counts so
all cores run one program). Each core:

  P1: one fused bf16 matmul per 128 nodes (lhsT = host-pretransposed
      X^T slice, rhs = [Ws@a2 | Ws@a1 | Ws]) writes a DRAM table row per
      node: [t, s, Wh x64, 1.0] bf16 = 134B in a 256B-stride row.
  P3: per 128-row block, one dma_gather (single_packet=False) fetches the
      256B table row of every edge's dst; one global cell gather fetches
      s_src per 8-edge cell, expanded to slots via a constant-matmul trick
      (sel8/E8) on the PE. w = exp(leaky(s+t)) on ACT; the one-hot-with-
      weight ohw = (iota32 == srel)*w (bf16) feeds PE accumulation
      acc[W:W+32, :] += ohw_c.T @ [Wh_dst | 1]_c over 32-aligned windows
      (PE tile_position), with slots padded to 128-chunks at 32-row
      sub-block boundaries so every chunk fits one window. Epilogue
      elu(U/Z) runs on ACT+Pool mid-stream (never blocking the in-order
      DVE queue on this block's matmuls) and on DVE for the final blocks.

Scheduling: engines are load-balanced (copies alternate DVE/ACT, memsets
and epilogue pieces on Pool/GPSIMD); DMA issue overhead is amortized
(~630ns/dma_start on shared HWDGE, 994ns fixed SWDGE desc-gen per gather
call); block 0's gather is split so a small piece leads (short desc-gen
after the last table write) and the last block's gather is split so the
critical chain after the final transfer covers only 16 chunks. The span
is DMA-bound: ~163us of HBM traffic (gathers dominate: 256B minimum
random-access granularity x ~86K slots), ~176us predicted total.
"""
import sys

if "/opt/trn_rl_repo" not in sys.path:
    sys.path.insert(0, "/opt/trn_rl_repo")

from contextlib import ExitStack

import numpy as np

import concourse.bass as bass
import concourse.tile as tile
from concourse import bacc, mybir
from concourse.bass_utils import run_bass_kernel_spmd
from concourse.masks import make_identity

N = 16384          # nodes
F = 128            # input features
D = 64             # embedding dim
NCORES = 8
R = N // NCORES    # rows per core (2048)
NB = R // 128      # 128-row blocks per core (16)
NSB = R // 32      # 32-row sub-blocks per core (64)
TW = 64            # table row width in f32 slots (256 bytes)
dt = mybir.dt


# ---------------------------------------------------------------- host prep
def _prep_edges(A):
    """Dedup edges, lay slots out 16-aligned per row, padded to 128-slot
    chunks at 32-row sub-block boundaries, chunk counts uniform across cores.
    Returns per-core arrays + shared layout metadata."""
    src_all = np.asarray(A[0], dtype=np.int64)
    dst_all = np.asarray(A[1], dtype=np.int64)
    keys = np.unique(src_all * N + dst_all)     # dedup + sort by (src, dst)
    src = (keys // N).astype(np.int32)
    dst = (keys % N).astype(np.int32)

    deg = np.bincount(src, minlength=N)
    assert deg.min() > 0, "empty rows present; kernel assumes deg >= 1"
    deg8 = ((deg + 7) // 8) * 8                  # 8-slot cells
    # sub-block = 32 consecutive rows; uniform chunk count across cores
    sb = np.arange(N) >> 5                       # global sub-block id
    cnt = np.bincount(sb, weights=deg8, minlength=N // 32).astype(np.int64)
    cnt = cnt.reshape(NCORES, NSB)
    Kb = np.maximum((cnt.max(axis=0) + 127) // 128, 1)   # [NSB] chunks/sub-blk
    SK = int(Kb.sum())                           # chunks per core
    S = SK * 128                                 # slots per core
    offs = np.concatenate([[0], np.cumsum(Kb)]) * 128    # slot offset / sub-blk
    ncell = S // 8
    SC = ((ncell + 127) // 128) * 128            # padded cells (single gather)

    row_start = np.concatenate([[0], np.cumsum(deg)])

    dsti = np.zeros((NCORES, S), np.int16)       # table idx per slot (pad->0)
    srel = np.full((NCORES, S), -1.0, np.float32)  # row - 32*sb, -1 = pad
    cellsrc = np.zeros((NCORES, SC), np.int16)   # src node per 16-slot cell
    for c in range(NCORES):
        for b in range(NSB):
            rows = np.arange((c * NSB + b) * 32, (c * NSB + b) * 32 + 32)
            pos = offs[b]
            for r in rows:
                dcnt = int(deg[r])
                lo = row_start[r]
                dsti[c, pos:pos + dcnt] = dst[lo:lo + dcnt]
                srel[c, pos:pos + dcnt] = float(r & 31)
                nc8 = int(deg8[r])
                cellsrc[c, pos // 8:(pos + nc8) // 8] = r
                pos += nc8
            assert pos <= offs[b + 1]

    def wrap(x):
        n = x.shape[0]
        w = x.reshape(n // 16, 16).T             # [16, n/16]
        return np.tile(w, (8, 1)).copy()         # [128, n/16]

    # per-128-row-block metadata
    K4, goffs, wins, segs = [], [], [], []
    g = 0
    for blk in range(NB):
        k4 = int(Kb[4 * blk:4 * blk + 4].sum())
        K4.append(k4)
        goffs.append(g)
        w = []
        for j in range(4):
            w += [32 * j] * int(Kb[4 * blk + j])
        wins.append(w)
        # rhsm segments: cell-chunk m covers s_ps cols 8m+cl for cl in [0,8)
        sg = []
        for m in range(g // 8, (g + k4 + 7) // 8):
            cl0 = max(0, g - 8 * m)
            cl1 = min(8, g + k4 - 8 * m)
            if cl1 > cl0:
                sg.append((m, cl0, cl1))
        segs.append(sg)
        g += k4
    assert g == SK

    cores = []
    for c in range(NCORES):
        cores.append({
            "dsti": wrap(dsti[c]),
            "celli": wrap(cellsrc[c]),
            "srel": srel[c].reshape(SK, 128).T.copy(),   # [128, SK]
        })
    meta = dict(S=S, SK=SK, SC=SC, K4=K4, goffs=goffs, wins=wins, segs=segs)
    return cores, meta


# ---------------------------------------------------------------- device IR
def _build(meta):
    S, SK, SC = meta["S"], meta["SK"], meta["SC"]
    K4, goffs, wins, segs = (meta["K4"], meta["goffs"], meta["wins"],
                             meta["segs"])
    CM = SC // 128
    nc = bacc.Bacc("TRN2", target_bir_lowering=False, debug=False,
                   enable_asserts=False, num_devices=NCORES,
                   num_swdge_queues=4)
    X_d = nc.dram_tensor("Xbt", [F, N], dt.bfloat16, kind="ExternalInput").ap()
    wsv_d = nc.dram_tensor("wsv", [128, D + 2], dt.float32,
                           kind="ExternalInput").ap()
    dsti_d = nc.dram_tensor("dsti", [128, S // 16], dt.int16,
                            kind="ExternalInput").ap()
    celli_d = nc.dram_tensor("celli", [128, SC // 16], dt.int16,
                             kind="ExternalInput").ap()
    srel_d = nc.dram_tensor("srel", [128, SK], dt.float32,
                            kind="ExternalInput").ap()
    sel8_d = nc.dram_tensor("sel8", [128, 8], dt.float32,
                            kind="ExternalInput").ap()
    E8_d = nc.dram_tensor("E8", [128, 128], dt.float32,
                          kind="ExternalInput").ap()
    out_d = nc.dram_tensor("out", [R, D], dt.float32, kind="ExternalOutput").ap()

    with tile.TileContext(nc) as tc, ExitStack() as ctx:
        cpool = ctx.enter_context(tc.tile_pool(name="const", bufs=1))
        dram = ctx.enter_context(tc.tile_pool(name="dram", bufs=1, space="DRAM"))
        twpool = ctx.enter_context(tc.tile_pool(name="tw", bufs=4))
        # PSUM budget (8 banks): wh 1x2, s 1x3, acc 1x3
        ps_wh = ctx.enter_context(tc.tile_pool(name="ps_wh", bufs=3, space="PSUM"))
        ps_se = ctx.enter_context(tc.tile_pool(name="ps_se", bufs=2, space="PSUM"))
        ps_ac = ctx.enter_context(tc.tile_pool(name="ps_ac", bufs=3, space="PSUM"))
        gpool = ctx.enter_context(tc.tile_pool(name="gat", bufs=5))
        dpool = ctx.enter_context(tc.tile_pool(name="d", bufs=4))
        opool = ctx.enter_context(tc.tile_pool(name="ohw", bufs=4))
        wpool = ctx.enter_context(tc.tile_pool(name="w", bufs=3))
        epool = ctx.enter_context(tc.tile_pool(name="ep", bufs=3))

        tabTW = dram.tile([N, TW], dt.float32)

        # ---- constants
        iota_i = cpool.tile([128, 32], dt.int32)
        nc.gpsimd.iota(iota_i[:], pattern=[[1, 32]], base=0,
                       channel_multiplier=0)
        iota32 = cpool.tile([128, 32], dt.bfloat16)
        nc.vector.tensor_copy(iota32[:], iota_i[:])
        wsv_f = cpool.tile([128, D + 2], dt.float32)
        nc.sync.dma_start(wsv_f[:], wsv_d)
        wsv = cpool.tile([128, D + 2], dt.bfloat16)
        nc.scalar.copy(wsv[:], wsv_f[:])

        # ---- P1: table row j (bf16) = [t, s, Wh x64, 1, pad] for all
        # nodes. X arrives pre-transposed and column-permuted from the host
        # (XbT[:, q*2048 + m*128 + i] = X[q*2048 + i*16 + m, :]) so the Wh
        # matmul needs no on-chip transpose and lhsT slices are contiguous.
        # Node n = q*2048 + p*16 + k: per partition 16 consecutive table
        # rows -> 4KB contiguous write descriptors.
        xbt = cpool.tile([128, N], dt.bfloat16)
        for q in range(8):
            # sliced loads: group q's matmuls unblock as its slice arrives
            nc.sync.dma_start(xbt[:, q * 2048:(q + 1) * 2048],
                              X_d[:, q * 2048:(q + 1) * 2048])
        tab_v = tabTW[:].rearrange("(q p k) w -> p q k w", p=128, k=16)
        for q in range(8):
            tw = twpool.tile([128, 16, 64], dt.float32)
            twb = tw[:].bitcast(dt.bfloat16)            # [128, 16, 128]
            nc.gpsimd.memset(twb[:, :, 66:67], 1.0)     # the [.. | 1] column
            for h in range(4):
                wh_ps = ps_wh.tile([128, 4, D + 2], dt.float32, space="PSUM",
                                   tag="wh")
                for k in range(4):
                    m = q * 16 + 4 * h + k
                    nc.tensor.matmul(wh_ps[:, k, :],
                                     lhsT=xbt[:, m * 128:(m + 1) * 128],
                                     rhs=wsv[:], start=True, stop=True)
                sl = slice(4 * h, 4 * h + 4)
                if h % 2 == 0:
                    nc.scalar.copy(twb[:, sl, 0:66], wh_ps[:])
                else:
                    nc.vector.tensor_copy(twb[:, sl, 0:66], wh_ps[:])
            # write only the 134B used per row; the DRAM pad is gathered
            # but never read by compute
            nc.scalar.dma_start(tab_v[:, q].bitcast(dt.bfloat16)[:, :, 0:67],
                                twb[:, :, 0:67])
        # P3 index/const loads: issued behind the X loads on the SP queue so
        # they don't delay P1's first transfers
        dsti_t = cpool.tile([128, S // 16], dt.int16)
        nc.sync.dma_start(dsti_t[:], dsti_d)
        celli_t = cpool.tile([128, SC // 16], dt.int16)
        nc.sync.dma_start(celli_t[:], celli_d)
        srel_f = cpool.tile([128, SK], dt.float32)
        nc.sync.dma_start(srel_f[:], srel_d)
        srel_t = cpool.tile([128, SK], dt.bfloat16)
        nc.vector.tensor_copy(srel_t[:], srel_f[:])
        sel8_t = cpool.tile([128, 8], dt.float32)
        nc.sync.dma_start(sel8_t[:], sel8_d)
        E8_t = cpool.tile([128, 128], dt.float32)
        nc.sync.dma_start(E8_t[:], E8_d)

        # ---- P3 prologue + per-block pipeline. Block 0 is split: a small
        # head gather leads (short desc-gen -> first transfer starts right
        # after the last table write) and a small tail gather is scheduled
        # after all other gathers, so the critical chain after the final DMA
        # covers only SPLIT chunks instead of a whole block.
        tab_ap = tabTW[:]
        SPLIT = 16

        def issue_gather(b, lo, hi):
            goff = goffs[b]
            gat = gpool.tile([128, hi - lo, TW], dt.float32, tag="gat")
            nc.gpsimd.dma_gather(
                out_ap=gat[:], in_ap=tab_ap,
                idxs_ap=dsti_t[:, (goff + lo) * 8:(goff + hi) * 8],
                num_idxs=(hi - lo) * 128, num_idxs_reg=(hi - lo) * 128,
                elem_size=TW, queue_num=0, single_packet=False)
            return gat

        gat0a = issue_gather(0, 0, SPLIT)
        gat1 = issue_gather(1, 0, K4[1])
        cellv = cpool.tile([128, CM, TW], dt.float32)
        nc.gpsimd.dma_gather(
            out_ap=cellv[:], in_ap=tab_ap, idxs_ap=celli_t[:],
            num_idxs=SC, num_idxs_reg=SC, elem_size=TW,
            queue_num=0, single_packet=False)
        cellb = cellv[:].bitcast(dt.bfloat16)            # [128, CM, 128]
        rhsm = cpool.tile([128, CM, 8], dt.float32)
        nc.vector.tensor_tensor(
            out=rhsm[:],
            in0=sel8_t[:, None, :].to_broadcast([128, CM, 8]),
            in1=cellb[:, :, 1:2].to_broadcast([128, CM, 8]),
            op=mybir.AluOpType.mult)

        out_v = out_d.rearrange("(b p) d -> p b d", p=128)   # [128, NB, D]
        accs, d_ts, s_pss = {}, {}, {}

        def block_head(b):
            K = K4[b]
            goff = goffs[b]
            s_ps = ps_se.tile([128, K], dt.float32, space="PSUM", tag="se")
            for (m, cl0, cl1) in segs[b]:
                nc.tensor.matmul(
                    s_ps[:, 8 * m + cl0 - goff:8 * m + cl1 - goff],
                    lhsT=E8_t[:], rhs=rhsm[:, m, cl0:cl1],
                    start=True, stop=True)
            d_t = dpool.tile([128, K, 32], dt.bfloat16)
            nc.vector.tensor_tensor(
                out=d_t[:],
                in0=iota32[:, None, :].to_broadcast([128, K, 32]),
                in1=srel_t[:, goff:goff + K, None]
                    .to_broadcast([128, K, 32]),
                op=mybir.AluOpType.subtract)
            acc = ps_ac.tile([128, D + 1], dt.float32, space="PSUM",
                             tag="ac")
            nc.vector.memzero(acc[:])
            accs[b], d_ts[b], s_pss[b] = acc, d_t, s_ps

        def block_body(b, lo, hi, gat, stop):
            K = hi - lo
            acc, d_t, s_ps = accs[b], d_ts[b], s_pss[b]
            gb = gat[:].bitcast(dt.bfloat16)            # [128, K, 128]
            e_t = wpool.tile([128, K], dt.float32, tag="e")
            nc.vector.tensor_add(e_t[:], s_ps[:, lo:hi], gb[:, :, 0])
            lk = wpool.tile([128, K], dt.float32, tag="lk")
            nc.vector.scalar_tensor_tensor(
                out=lk[:], in0=e_t[:], scalar=0.15,
                op0=mybir.AluOpType.mult,
                in1=e_t[:], op1=mybir.AluOpType.max)
            w_t = wpool.tile([128, K], dt.bfloat16, tag="wt")
            nc.scalar.activation(w_t[:], lk[:],
                                 mybir.ActivationFunctionType.Exp)
            ohw = opool.tile([128, K, 32], dt.bfloat16)
            nc.vector.scalar_tensor_tensor(
                out=ohw[:], in0=d_t[:, lo:hi, :], scalar=0.0,
                op0=mybir.AluOpType.is_equal,
                in1=w_t[:, :, None].to_broadcast([128, K, 32]),
                op1=mybir.AluOpType.mult)
            for c in range(K):
                W = wins[b][lo + c]
                nc.tensor.matmul(acc[W:W + 32, :], lhsT=ohw[:, c, :],
                                 rhs=gb[:, c, 2:2 + D + 1],
                                 start=False, stop=(stop and c == K - 1),
                                 skip_group_check=True,
                                 tile_position=(0, W))

        def block_epilogue_dve(b):
            # end-game variant: DVE is drained by the time the last blocks'
            # epilogues run, while Act/Pool host the final w/ohw chain
            acc = accs[b]
            zr = epool.tile([128, 1], dt.float32, tag="zr")
            nc.vector.reciprocal(zr[:], acc[:, D:D + 1])
            x = epool.tile([128, D], dt.float32, tag="x")
            nc.vector.tensor_scalar_mul(x[:], acc[:, 0:D], zr[:])
            em = epool.tile([128, D], dt.float32, tag="em")
            nc.scalar.activation(em[:], x[:],
                                 mybir.ActivationFunctionType.Exp)
            rl = epool.tile([128, D], dt.float32, tag="rl")
            nc.vector.tensor_scalar_max(rl[:], x[:], 0.0)
            qm = epool.tile([128, D], dt.float32, tag="qm")
            nc.vector.tensor_scalar_min(qm[:], em[:], 1.0)
            ob = epool.tile([128, D], dt.float32, tag="ob")
            nc.vector.scalar_tensor_tensor(
                out=ob[:], in0=qm[:], scalar=-1.0,
                op0=mybir.AluOpType.add, in1=rl[:], op1=mybir.AluOpType.add)
            nc.sync.dma_start(out_v[:, b, :], ob[:])

        def block_epilogue(b):
            # out = elu(U/Z) = relu(x) + (min(exp(x),1) - 1) on ACT+Pool
            # only: the DVE in-order queue must never host an op that waits
            # on this block's matmuls. Z > 0 always (deg >= 1, w > 0).
            acc = accs[b]
            uz = epool.tile([128, D + 1], dt.float32, tag="uz")
            nc.scalar.copy(uz[:], acc[:])
            x = epool.tile([128, D], dt.float32, tag="x")
            nc.gpsimd.normalize_recip(x[:], uz[:, 0:D], uz[:, D:D + 1])
            em = epool.tile([128, D], dt.float32, tag="em")
            nc.scalar.activation(em[:], x[:],
                                 mybir.ActivationFunctionType.Exp)
            rl = epool.tile([128, D], dt.float32, tag="rl")
            nc.scalar.activation(rl[:], x[:],
                                 mybir.ActivationFunctionType.Relu)
            qm = epool.tile([128, D], dt.float32, tag="qm")
            nc.gpsimd.tensor_scalar_min(qm[:], em[:], 1.0)
            q1 = epool.tile([128, D], dt.float32, tag="q1")
            nc.gpsimd.tensor_scalar_add(q1[:], qm[:], -1.0)
            ob = epool.tile([128, D], dt.float32, tag="ob")
            nc.gpsimd.tensor_add(ob[:], q1[:], rl[:])
            nc.sync.dma_start(out_v[:, b, :], ob[:])

        block_head(0)
        block_body(0, 0, SPLIT, gat0a, stop=False)
        gat0m = issue_gather(0, SPLIT, K4[0])
        block_body(0, SPLIT, K4[0], gat0m, stop=True)
        block_head(1)
        block_body(1, 0, K4[1], gat1, stop=True)
        # epilogue(b) is emitted two blocks later: any wait it contributes
        # to an engine queue head has long been satisfied by then
        for b in range(2, NB - 1):
            gat = issue_gather(b, 0, K4[b])
            block_head(b)
            block_body(b, 0, K4[b], gat, stop=True)
            block_epilogue(b - 2)
        # last block: second small gather piece keeps the critical chain
        # after the final transfer short
        bl = NB - 1
        KL, E = K4[bl], 16
        gatLa = issue_gather(bl, 0, KL - E)
        block_head(bl)
        block_body(bl, 0, KL - E, gatLa, stop=False)
        block_epilogue(NB - 3)
        gatLb = issue_gather(bl, KL - E, KL)
        block_body(bl, KL - E, KL, gatLb, stop=True)
        block_epilogue_dve(NB - 2)
        block_epilogue_dve(NB - 1)
    nc.compile()
    return nc


_cache = {}


def _get_program(meta):
    key = (meta["S"], meta["SC"], tuple(meta["K4"]))
    if key not in _cache:
        _cache[key] = _build(meta)
    return _cache[key]


def make_in_maps(A, X, Ws, a):
    """Host-side sharding: returns (nc, in_maps)."""
    import ml_dtypes
    XbT = np.asarray(X, dtype=np.float32).astype(ml_dtypes.bfloat16).T
    # permute columns to the n = q*2048 + p*16 + k consumption order
    XbT = np.ascontiguousarray(
        XbT.reshape(128, 8, 128, 16).transpose(0, 1, 3, 2).reshape(128, N))
    Ws = np.asarray(Ws, dtype=np.float32)
    a = np.asarray(a, dtype=np.float32).reshape(2 * D)
    v1 = (Ws.astype(np.float64) @ a[:D].astype(np.float64)).astype(np.float32)
    v2 = (Ws.astype(np.float64) @ a[D:].astype(np.float64)).astype(np.float32)
    wsv = np.concatenate([v2[:, None], v1[:, None], Ws], axis=1)  # [128, 66]
    q = np.arange(128)
    sel8 = (q[:, None] // 16 == np.arange(8)[None, :]).astype(np.float32)
    E8 = (q[:, None] % 16 == q[None, :] // 8).astype(np.float32)
    cores, meta = _prep_edges(A)
    nc = _get_program(meta)
    in_maps = [
        {"Xbt": XbT, "wsv": wsv, "sel8": sel8, "E8": E8,
         "dsti": c["dsti"], "celli": c["celli"], "srel": c["srel"]}
        for c in cores
    ]
    return nc, in_maps


def kernel(A, X, Ws, a):
    nc, in_maps = make_in_maps(A, X, Ws, a)
    res = run_bass_kernel_spmd(nc, in_maps, core_ids=list(range(NCORES)),
                               trace=False)
    return np.concatenate([r["out"] for r in res.results], axis=0)
